# revision 19
# baseline (speedup 1.0000x reference)
"""DenseGGNN (gnn_message_passing) Trainium2 Bass kernel.

Math per layer i (per batch):
    s  = A^T @ h                    # [N, C], A binary adjacency
    gx = s @ (W_i @ w_ih_i^T)       # fused:  ((A^T h) W) @ w_ih^T
    gh = h @ w_hh_i^T
    r  = sigmoid(gx_r + gh_r + b_r);  zc = 1 - z = sigmoid(-(gx_z + gh_z + b_z))
    n  = tanh(gx_n + b_in + r * (gh_n + b_hn))
    h' = h + zc * (n - h)

Single-pass fp16 matmuls throughout (the 2e-2 rel-err gate leaves ~40x
headroom over the fp16 rounding floor; measured final err ~1e-3).  PSUM
accumulates fp32; the GRU elementwise runs mostly fp16 and is spread
across ACT (sigmoid/tanh), DVE (fused add-chains, fp16 2x mode) and
Pool (copies, muls) so no single engine exceeds ~60% of PE time.

Device layout: state is feature-major hT [C=128 part, N=1024] fp16; the
s-matmul consumes a node-major fp16 copy (h_nm) produced by a DMA-xbar
transpose each layer.  x arrives from the host pre-transposed in BOTH
layouts, so the device does no initialization work.  The output leaves
feature-major [C, N] fp32 and the host transposes it back.

Sharding: batch (32) split across 8 cores, 4 batches/core, weights
replicated; no cross-core communication.
"""

from contextlib import ExitStack, nullcontext

import numpy as np

import concourse.bass as bass
import concourse.bacc as bacc
import concourse.tile as tile
import concourse.mybir as mybir
from concourse.bass_utils import run_bass_kernel_spmd

B, N, C, L = 32, 1024, 128, 4
NCORES = 8
BPC = B // NCORES          # batches per core
P = 128                    # partitions
NT = N // P                # node tiles (8)
HALF = 512                 # psum-bank-sized column chunk

F32 = mybir.dt.float32
F16 = mybir.dt.float16
AF = mybir.ActivationFunctionType
ALU = mybir.AluOpType

_PROGRAM_CACHE = {}


def _build_program(reps: int = 1, loop_reps: int = 1) -> bass.Bass:
    # reps > 1 re-emits the whole body back-to-back in one NEFF;
    # loop_reps > 1 wraps the body in a hardware For_i loop.  Both are
    # benchmarking aids (wall-time slope isolates per-iteration device
    # time from the axon dispatch overhead).
    nc = bacc.Bacc()

    xnm_d = nc.declare_dram_parameter("xnm", [BPC, P, NT, C], F16, isOutput=False)
    xlo_d = nc.declare_dram_parameter("xlo", [BPC, P, NT, C], F16, isOutput=False)
    xT_d = nc.declare_dram_parameter("xT", [BPC, C, N], F16, isOutput=False)
    adj_d = nc.declare_dram_parameter("adj", [BPC, N, N], F16, isOutput=False)
    wch_d = nc.declare_dram_parameter("wch", [C, L, 3, C], F16, isOutput=False)
    wcl_d = nc.declare_dram_parameter("wcl", [C, L, 3, C], F16, isOutput=False)
    whh_d = nc.declare_dram_parameter("whh", [C, L, 3, C], F16, isOutput=False)
    bias_d = nc.declare_dram_parameter("bias", [C, L, 4], F32, isOutput=False)
    y_d = nc.declare_dram_parameter("y", [BPC, C, N], F32, isOutput=True)

    with tile.TileContext(nc) as tc, ExitStack() as ctx:
        consts = ctx.enter_context(tc.tile_pool(name="consts", bufs=1))
        adj_pool = ctx.enter_context(tc.tile_pool(name="adjp", bufs=1))
        xlo_pool = ctx.enter_context(tc.tile_pool(name="xlo", bufs=1))
        hnm_pool = ctx.enter_context(tc.tile_pool(name="hnm", bufs=2))
        hT_pool = ctx.enter_context(tc.tile_pool(name="hT", bufs=2))
        s_pool = ctx.enter_context(tc.tile_pool(name="s16", bufs=1))
        nz_pool = ctx.enter_context(tc.tile_pool(name="nz", bufs=1))
        ew_pool = ctx.enter_context(tc.tile_pool(name="ew", bufs=2))
        de_pool = ctx.enter_context(tc.tile_pool(name="de", bufs=2))
        y_pool = ctx.enter_context(tc.tile_pool(name="yp", bufs=2))
        ps_s = ctx.enter_context(tc.tile_pool(name="ps_s", bufs=1, space="PSUM"))
        ps_g = ctx.enter_context(tc.tile_pool(name="ps_g", bufs=7, space="PSUM"))

        def wslice(w, i, g):
            return w[:, (i * 3 + g) * C:(i * 3 + g + 1) * C]

        def bslice(i, k):
            return bias[:, i * 4 + k:i * 4 + k + 1]

        loop_cm = (tc.For_i(0, loop_reps, 1, hint_engines=(mybir.EngineType.PE,))
                   if loop_reps > 1 else nullcontext())
        with loop_cm:
          for _rep in range(reps):
            # ---- input loads -------------------------------------------------
            # adjacency rides the SP HWDGE ring (it also carries the xbar
            # transposes + stores later); x + weights ride the ACT ring.
            # Load order follows the layer-0 consumption deadlines:
            # ACT ring: xnm0 -> weights -> xnm1-3 -> xT0-3 (xT only needed
            # once the gates start); SP ring: all adjacency (deadline: last
            # s-matmul of layer 0).
            adj_sb = []
            hnm = [None] * BPC
            xlo = [None] * BPC
            hT = [None] * BPC
            for b in range(BPC):
                a = adj_pool.tile([P, NT, N], F16, tag=f"adj{b}")
                # adj was cast to fp16 on the host (exact for 0/1 entries).
                # Two chunks so the first j-tiles land early; j = t*128+p.
                src = adj_d[b].rearrange("(t p) n -> p t n", p=P)
                nc.sync.dma_start(a[:, 0:NT // 2, :], src[:, 0:NT // 2, :])
                nc.sync.dma_start(a[:, NT // 2:, :], src[:, NT // 2:, :])
                adj_sb.append(a)
                hn = hnm_pool.tile([P, NT, C], F16, tag=f"hnm{b}")
                nc.scalar.dma_start(hn[:], xnm_d[b])
                hnm[b] = hn
                xl = xlo_pool.tile([P, NT, C], F16, tag=f"xlo{b}")
                nc.scalar.dma_start(xl[:], xlo_d[b])
                xlo[b] = xl
                if b == 0:
                    wch = consts.tile([P, L * 3 * C], F16)
                    nc.scalar.dma_start(wch[:],
                                        wch_d.rearrange("c l g d -> c (l g d)"))
                    wcl = consts.tile([P, L * 3 * C], F16)
                    nc.scalar.dma_start(wcl[:],
                                        wcl_d.rearrange("c l g d -> c (l g d)"))
                    whh = consts.tile([P, L * 3 * C], F16)
                    nc.scalar.dma_start(whh[:],
                                        whh_d.rearrange("c l g d -> c (l g d)"))
                    bias = consts.tile([P, L * 4], F32)
                    nc.scalar.dma_start(bias[:],
                                        bias_d.rearrange("c l k -> c (l k)"))
            for b in range(BPC):
                ht = hT_pool.tile([P, N], F16, tag=f"hT{b}")
                nc.scalar.dma_start(ht[:], xT_d[b])
                hT[b] = ht

            # ---- layers ------------------------------------------------------
            s_sb = [None] * BPC
            for i in range(L):
                last_layer = i == L - 1
                # layer-0 perturbations amplify ~200x through the remaining
                # GRU layers, so layer 0 runs the s-matmul split (x = hi + lo,
                # both fp16 from the host); later layers single-pass fp16.
                # The fused gate weight keeps an fp16 lo-correction pass
                # (wcl) on layers 0-2 for the same reason.
                wcl_layer = i < L - 1

                def do_s(b):
                    # s = (A^T h)^T accumulated in psum, fp16 to sbuf
                    stats = (hnm[b], xlo[b]) if i == 0 else (hnm[b],)
                    s16 = s_pool.tile([P, N], F16, tag=f"s{b}")
                    for half in range(2):
                        hs = slice(half * HALF, (half + 1) * HALF)
                        ps = ps_s.tile([P, HALF], F32, tag="ps_s")
                        for ti, hnmt in enumerate(stats):
                            for j in range(NT):
                                nc.tensor.matmul(
                                    ps[:],
                                    lhsT=hnmt[:, j, :],
                                    rhs=adj_sb[b][:, j, hs],
                                    start=(ti == 0 and j == 0),
                                    stop=(ti == len(stats) - 1 and j == NT - 1),
                                )
                        nc.vector.tensor_copy(s16[:, hs], ps[:])
                    s_sb[b] = s16

                def do_gates(b):
                    n16 = nz_pool.tile([P, N], F16, tag=f"n{b}")
                    zc16 = nz_pool.tile([P, N], F16, tag=f"z{b}")
                    if last_layer:
                        y32 = y_pool.tile([P, N], F32, tag="y")
                    else:
                        hTn = hT_pool.tile([P, N], F16, tag=f"hT{b}")
                        hn = hnm_pool.tile([P, NT, C], F16, tag=f"hnm{b}")
                    for half in range(2):
                        hs = slice(half * HALF, (half + 1) * HALF)
                        pr = ps_g.tile([P, HALF], F32, tag="psg")
                        pz = ps_g.tile([P, HALF], F32, tag="psg")
                        pxn = ps_g.tile([P, HALF], F32, tag="psg")
                        phn = ps_g.tile([P, HALF], F32, tag="psg")
                        for g, pg in ((0, pr), (1, pz)):
                            nc.tensor.matmul(pg[:], lhsT=wslice(wch, i, g),
                                             rhs=s_sb[b][:, hs],
                                             start=True, stop=False)
                            if wcl_layer:
                                nc.tensor.matmul(pg[:], lhsT=wslice(wcl, i, g),
                                                 rhs=s_sb[b][:, hs],
                                                 start=False, stop=False)
                            nc.tensor.matmul(pg[:], lhsT=wslice(whh, i, g),
                                             rhs=hT[b][:, hs],
                                             start=False, stop=True)
                        nc.tensor.matmul(pxn[:], lhsT=wslice(wch, i, 2),
                                         rhs=s_sb[b][:, hs], start=True,
                                         stop=not wcl_layer)
                        if wcl_layer:
                            nc.tensor.matmul(pxn[:], lhsT=wslice(wcl, i, 2),
                                             rhs=s_sb[b][:, hs],
                                             start=False, stop=True)
                        nc.tensor.matmul(phn[:], lhsT=wslice(whh, i, 2),
                                         rhs=hT[b][:, hs], start=True, stop=True)

                        r16 = ew_pool.tile([P, HALF], F16, tag="r")
                        nc.scalar.activation(r16[:], pr[:], AF.Sigmoid,
                                             bias=bslice(i, 0))
                        nc.scalar.activation(zc16[:, hs], pz[:], AF.Sigmoid,
                                             bias=bslice(i, 1), scale=-1.0)
                        t16 = ew_pool.tile([P, HALF], F16, tag="t")
                        nc.vector.scalar_tensor_tensor(t16[:], phn[:], bslice(i, 3),
                                                       r16[:],
                                                       op0=ALU.add, op1=ALU.mult)
                        u16 = ew_pool.tile([P, HALF], F16, tag="u")
                        nc.vector.scalar_tensor_tensor(u16[:], pxn[:], bslice(i, 2),
                                                       t16[:],
                                                       op0=ALU.add, op1=ALU.add)
                        nc.scalar.activation(n16[:, hs], u16[:], AF.Tanh)

                        # per-half state update on Pool; the transpose for
                        # the next layer streams out as each half finishes
                        d16 = de_pool.tile([P, HALF], F16, tag="d")
                        nc.gpsimd.tensor_sub(d16[:], n16[:, hs], hT[b][:, hs])
                        e16 = de_pool.tile([P, HALF], F16, tag="e")
                        nc.gpsimd.tensor_mul(e16[:], zc16[:, hs], d16[:])
                        if last_layer:
                            nc.vector.tensor_add(y32[:, hs], hT[b][:, hs], e16[:])
                            nc.sync.dma_start(y_d[b][:, hs], y32[:, hs])
                        else:
                            nc.gpsimd.tensor_add(hTn[:, hs], hT[b][:, hs], e16[:])
                            ht_sl = slice(half * (NT // 2), (half + 1) * (NT // 2))
                            nc.scalar.dma_start(out=hn[:, ht_sl, :],
                                                in_=hTn[:, hs], transpose=True)
                    if not last_layer:
                        hT[b] = hTn
                        hnm[b] = hn

                # interleave: g(b) runs on the PE while s(b+2) streams, so
                # the elementwise/transpose chain for batch b completes long
                # before the next layer's s(b) needs it
                do_s(0)
                do_s(1)
                do_gates(0)
                do_s(2)
                do_gates(1)
                do_s(3)
                do_gates(2)
                do_gates(3)

    nc.finalize()
    return nc


def _prep_weights(weight, w_ih, w_hh, b_ih, b_hh):
    weight = np.asarray(weight, np.float32)
    w_ih = np.asarray(w_ih, np.float32)
    w_hh = np.asarray(w_hh, np.float32)
    b_ih = np.asarray(b_ih, np.float32)
    b_hh = np.asarray(b_hh, np.float32)

    # fused input-gate weight: gx = s @ (W @ w_ih^T), as [C, L, 3, C]
    wc = np.einsum("lcd,lgd->lcg", weight, w_ih)          # [L, C, 3C]
    wch = wc.astype(np.float16)
    wcl = (wc - wch.astype(np.float32)).astype(np.float16)
    whh_t = np.transpose(w_hh, (0, 2, 1)).astype(np.float16)  # [L, C, 3C]

    def to_clgd(a):  # [L, C, 3C] -> [C, L, 3, C]
        return np.ascontiguousarray(
            np.transpose(a.reshape(L, C, 3, C), (1, 0, 2, 3)))

    bias = np.empty((C, L, 4), np.float32)
    bias[:, :, 0] = (b_ih[:, 0:C] + b_hh[:, 0:C]).T
    bias[:, :, 1] = -(b_ih[:, C:2 * C] + b_hh[:, C:2 * C]).T
    bias[:, :, 2] = b_ih[:, 2 * C:3 * C].T
    bias[:, :, 3] = b_hh[:, 2 * C:3 * C].T

    return to_clgd(wch), to_clgd(wcl), to_clgd(whh_t), bias


def kernel(x, adj, mask, weight, w_ih, w_hh, b_ih, b_hh, _run_kwargs=None):
    x = np.asarray(x, np.float32)
    # binary adjacency: fp16 is exact, halves the HBM traffic on device
    adj = np.asarray(adj, np.float32).astype(np.float16)
    mask = np.asarray(mask, np.float32)
    wch, wcl, whh, bias = _prep_weights(weight, w_ih, w_hh, b_ih, b_hh)

    x16 = x.astype(np.float16)
    xlo16 = (x - x16.astype(np.float32)).astype(np.float16)
    # node-major stationary tiles: [B, P, NT, C], partition-major so each
    # SBUF partition reads one contiguous 2KB run
    def to_nm(a):
        return np.ascontiguousarray(a.reshape(B, NT, P, C).transpose(0, 2, 1, 3))
    xnm = to_nm(x16)
    xlo = to_nm(xlo16)
    # feature-major state: [B, C, N]
    xT = np.ascontiguousarray(x16.transpose(0, 2, 1))

    if "nc" not in _PROGRAM_CACHE:
        _PROGRAM_CACHE["nc"] = _build_program()
    nc = _PROGRAM_CACHE["nc"]

    in_maps = []
    for c in range(NCORES):
        sl = slice(c * BPC, (c + 1) * BPC)
        in_maps.append({
            "xnm": np.ascontiguousarray(xnm[sl]),
            "xlo": np.ascontiguousarray(xlo[sl]),
            "xT": np.ascontiguousarray(xT[sl]),
            "adj": np.ascontiguousarray(adj[sl]),
            "wch": wch, "wcl": wcl, "whh": whh, "bias": bias,
        })

    res = run_bass_kernel_spmd(nc, in_maps, list(range(NCORES)),
                               **(_run_kwargs or {}))
    # y arrives feature-major [BPC, C, N]; transpose back on host
    y = np.concatenate([r["y"] for r in res.results], axis=0)
    y = np.ascontiguousarray(y.transpose(0, 2, 1))
    y = y * mask[:, :, None]
    if _run_kwargs:
        kernel.last_results = res
    return y.astype(np.float32)
